# revision 29
# baseline (speedup 1.0000x reference)
"""RBF kernel ridge regression inference on 8 Trainium2 NeuronCores.

out[q] = sum_t exp(-gamma * ||X[q] - T[t]||^2) * coef[t]
       = exp(-g*x2[q]) * sum_t exp(2g*dot[t,q] - g*y2[t]) * coef[t]

Train index on PSUM partitions, query index on the free axis, all GEMM data
in fp8:

  TensorE: dot GEMM in fp8 DoubleRow mode (256-deep contraction per matmul,
           2x PE throughput), 8 matmuls per [128,2048] psum tile, preceded
           by HAM-warmup matmuls that run while the input DMAs land; then a
           coef-matvec sweep over the resident fp8 exp tiles, 4 col-tiled
           matmuls at a time running concurrently in separate 32-column
           groups of the PE array, accumulating into four [1,512] S rows
           packed at partitions 0/32/64/96 of one psum bank.
  ScalarE: one fused exp per [128,2048] psum tile (bias = -g*y2[t] per
           partition, scale = 2g), writing fp8 et tiles that stay resident
           in SBUF (64KB/partition) until the matvec sweep.
  DVE:     row norms only (square+accumulate at 1 elem/cycle/lane), plus
           the tiny row-space epilogue multiply by exp(-g*x2[q]).
  DMA:     x/tt on the SP hardware queue (first-needed slices first),
           norm inputs on the gpsimd software queue; x2 column->row
           transpose bounced through DRAM under phase 1.

Sharding: 4 query-groups x 2 train-groups over 8 cores; host sums the two
train partials per query group (32KB of adds).
"""

import numpy as np
import ml_dtypes

GAMMA = 1.0
N_QUERY, N_TRAIN, D = 8192, 8192, 512
N_CORES = 8
P = 128
QG, TG = 4, 2             # query groups x train groups = 8 cores
QPC = N_QUERY // QG       # 2048 queries per core
TPC = N_TRAIN // TG       # 4096 train points per core
NQC = QPC // P            # 16 query chunks of 128
NTC = TPC // P            # 32 train chunks of 128
NDR = 2                   # DoubleRow contraction groups (2 x 256 = 512)
NPAIR = NTC // 2          # 16 train-chunk pairs for the DR matvec
NJ = QPC // 512           # 4 matvec/qcol slices

_CACHE = {}


def _build_program():
    from contextlib import ExitStack

    import concourse.bass as bass
    import concourse.mybir as mybir
    import concourse.tile as tile
    from concourse import bacc

    f32 = mybir.dt.float32
    bf16 = mybir.dt.bfloat16
    f8 = mybir.dt.float8e4
    AF = mybir.ActivationFunctionType
    MUL = mybir.AluOpType.mult
    DR = mybir.MatmulPerfMode.DoubleRow

    nc = bacc.Bacc(
        "TRN2", target_bir_lowering=False, debug=False, num_devices=N_CORES
    )

    # DRAM inputs (host-pre-laid-out; see make_in_maps)
    tt_d = [
        nc.dram_tensor(f"tt{g}", [P, NDR, TPC], f8, kind="ExternalInput").ap()
        for g in range(NDR)
    ]
    x_d = [
        nc.dram_tensor(f"x{g}", [P, NDR, QPC], f8, kind="ExternalInput").ap()
        for g in range(NDR)
    ]
    tr_d = [
        nc.dram_tensor(f"tr{b}", [P, 8, D], f8, kind="ExternalInput").ap()
        for b in range(TPC // (8 * P))
    ]
    xr_d = [
        nc.dram_tensor(f"xr{b}", [P, 8, D], f8, kind="ExternalInput").ap()
        for b in range(QPC // (8 * P))
    ]
    coef_d = nc.dram_tensor("coefc", [P, NTC], f8, kind="ExternalInput").ap()
    out_d = nc.dram_tensor("out", [QPC], f32, kind="ExternalOutput").ap()
    x2_d = nc.dram_tensor("x2_bounce", [QPC], f32).ap()   # internal scratch

    with tile.TileContext(nc) as tc, ExitStack() as ctx:
        res = ctx.enter_context(tc.tile_pool(name="res", bufs=1))
        stream = ctx.enter_context(tc.tile_pool(name="stream", bufs=2))

        # ---- resident GEMM operands ----
        tt_sb, x_sb = [], []
        for g in range(NDR):
            t = res.tile([P, NDR, QPC], f8, tag=f"x{g}", name=f"x{g}s")
            x_sb.append(t)
        for g in range(NDR):
            t = res.tile([P, NDR, TPC], f8, tag=f"tt{g}", name=f"tt{g}s")
            tt_sb.append(t)
        # first-needed slices first: one x j-slice per group + the first
        # 512 train columns, then the bulk
        for g in range(NDR):
            nc.sync.dma_start(x_sb[g][:, :, :512], x_d[g][:, :, :512])
        for g in range(NDR):
            nc.sync.dma_start(tt_sb[g][:, :, :512], tt_d[g][:, :, :512])
        for joff in range(512, QPC, 512):
            for g in range(NDR):
                nc.sync.dma_start(
                    x_sb[g][:, :, joff : joff + 512],
                    x_d[g][:, :, joff : joff + 512],
                )
        QTR = TPC // 4
        for g in range(NDR):
            nc.sync.dma_start(tt_sb[g][:, :, 512:QTR], tt_d[g][:, :, 512:QTR])

        # ---- norms: ny2 = -g*||T[t]||^2 (col, bias), nx2 = -g*||X[q]||^2 ----
        ny2 = res.tile([P, NTC], f32, tag="ny2")
        trt0 = stream.tile([P, 8, D], f8, tag="tr", name="trt0")
        nc.gpsimd.dma_start(trt0[:], tr_d[0])
        coefc = res.tile([P, NTC], f8, tag="coefc")
        nc.gpsimd.dma_start(coefc[:], coef_d)
        for h in range(1, 4):
            for g in range(NDR):
                sl = slice(h * QTR, (h + 1) * QTR)
                nc.sync.dma_start(tt_sb[g][:, :, sl], tt_d[g][:, :, sl])
        for b in range(len(tr_d)):
            if b == 0:
                trt = trt0
            else:
                trt = stream.tile([P, 8, D], f8, tag="tr")
                nc.gpsimd.dma_start(trt[:], tr_d[b])
            tscr = stream.tile([P, 8, D], bf16, tag="scr", name="tscr")
            for j in range(8):
                nc.vector.scalar_tensor_tensor(
                    tscr[:, j : j + 1, :],
                    trt[:, j : j + 1, :],
                    -GAMMA,
                    trt[:, j : j + 1, :],
                    MUL,
                    MUL,
                    accum_out=ny2[:, 8 * b + j : 8 * b + j + 1],
                )
        nx2 = res.tile([P, NQC], f32, tag="nx2")
        for b in range(len(xr_d)):
            xrt = stream.tile([P, 8, D], f8, tag="xr")
            nc.gpsimd.dma_start(xrt[:], xr_d[b])
            xscr = stream.tile([P, 8, D], bf16, tag="scr", name="xscr")
            for j in range(8):
                nc.vector.scalar_tensor_tensor(
                    xscr[:, j : j + 1, :],
                    xrt[:, j : j + 1, :],
                    -GAMMA,
                    xrt[:, j : j + 1, :],
                    MUL,
                    MUL,
                    accum_out=nx2[:, 8 * b + j : 8 * b + j + 1],
                )
        # x2 -> row layout early (scattered bounce hides under phase 1)
        nc.sync.dma_start(x2_d.rearrange("(c p) -> p c", p=P), nx2[:])
        nx2row = res.tile([1, QPC], f32, tag="nx2row")
        nc.sync.dma_start(nx2row[:], x2_d.rearrange("(a q) -> a q", a=1))

        # ---- phase 1: GEMM + exp, double-buffered [128,2048] psum tiles;
        # all 32 et tiles stay resident in SBUF for phase 2 ----
        ET = res.tile([P, NTC * QPC], f8, tag="ET")
        psq = ctx.enter_context(tc.tile_pool(name="psq", bufs=2, space="PSUM"))
        if True:
            # HAM warmup: dummy matmuls on the first-arrived slices run during
            # the remaining input DMAs so phase 1 starts at the full PE clock
            wu = psq.tile([P, QPC], f32, tag="ps", name="wu")
            for w in range(8):
                nc.tensor.matmul(
                    wu[:, (w % 4) * 512 : (w % 4) * 512 + 512],
                    tt_sb[0][:, :, :P],
                    x_sb[0][:, :, :512],
                    start=True,
                    stop=True,
                    perf_mode=DR,
                )
            for ti in range(NTC):
                ps = psq.tile([P, QPC], f32, tag="ps")
                for joff in range(0, QPC, 512):
                    for g in range(NDR):
                        nc.tensor.matmul(
                            ps[:, joff : joff + 512],
                            tt_sb[g][:, :, ti * P : (ti + 1) * P],
                            x_sb[g][:, :, joff : joff + 512],
                            start=(g == 0),
                            stop=(g == NDR - 1),
                            perf_mode=DR,
                        )
                nc.scalar.activation(
                    ET[:, ti * QPC : (ti + 1) * QPC],
                    ps[:],
                    AF.Exp,
                    bias=ny2[:, ti : ti + 1],
                    scale=2.0 * GAMMA,
                )

        ex2row = res.tile([1, QPC], f32, tag="ex2row")
        nc.scalar.activation(ex2row[:], nx2row[:], AF.Exp)
        S4 = psq.tile([P, 512], f32, tag="ps", name="S4")

        # ---- phase 2: coef-matvec sweep; per ti the 4 col-tiled fp8
        # matmuls run concurrently in separate 32-column groups ----
        for ti in range(NTC):
            for j in range(NJ):
                nc.tensor.matmul(
                    S4[32 * j : 32 * j + 1, :],
                    coefc[:, ti : ti + 1],
                    ET[:, ti * QPC + j * 512 : ti * QPC + (j + 1) * 512],
                    start=(ti == 0),
                    stop=(ti == NTC - 1),
                    tile_position=(0, 32 * j),
                )

        # ---- epilogue: out = exp(-g*x2) * S in row layout; S rows are
        # copied out of PSUM then gathered to partition 0 by SBUF-SBUF DMA ----
        s_sb = res.tile([P, 512], f32, tag="s_sb")
        srow = res.tile([1, QPC], f32, tag="srow")
        for j in range(NJ):
            if j % 2 == 0:
                nc.vector.tensor_copy(
                    s_sb[32 * j : 32 * j + 1, :], S4[32 * j : 32 * j + 1, :]
                )
            else:
                nc.scalar.copy(
                    s_sb[32 * j : 32 * j + 1, :], S4[32 * j : 32 * j + 1, :]
                )
            nc.sync.dma_start(
                srow[:, j * 512 : (j + 1) * 512], s_sb[32 * j : 32 * j + 1, :]
            )
        outrow = res.tile([1, QPC], f32, tag="outrow")
        nc.vector.tensor_mul(outrow[:], ex2row[:], srow[:])
        nc.sync.dma_start(out_d.rearrange("(a q) -> a q", a=1), outrow[:])

    nc.compile()
    return nc


def _get_program():
    if "nc" not in _CACHE:
        _CACHE["nc"] = _build_program()
    return _CACHE["nc"]


def make_in_maps(X, train_X, dual_coef):
    bf = ml_dtypes.bfloat16
    f8 = ml_dtypes.float8_e4m3

    def dr_layout(A):
        # A: [n, D] rows -> [g][P, 2, n] with d = 256*g + 128*i + p
        At = np.ascontiguousarray(A.T).reshape(NDR, 2, P, A.shape[0])
        return [np.ascontiguousarray(At[g].transpose(1, 0, 2)).astype(f8)
                for g in range(NDR)]

    def row_blocks(A):
        # A: [n, D] -> [n//(8P)][P, 8, D] with row = b*8P + j*P + p
        n = A.shape[0]
        R = A.reshape(n // (8 * P), 8, P, D)
        return [np.ascontiguousarray(R[b].transpose(1, 0, 2)).astype(f8)
                for b in range(n // (8 * P))]

    in_maps = []
    for c in range(N_CORES):
        i, j = c // TG, c % TG
        Xs = X[i * QPC : (i + 1) * QPC]
        Ts = train_X[j * TPC : (j + 1) * TPC]
        cs = dual_coef[j * TPC : (j + 1) * TPC]
        m = {}
        for g, arr in enumerate(dr_layout(Xs)):
            m[f"x{g}"] = arr
        for g, arr in enumerate(dr_layout(Ts)):
            m[f"tt{g}"] = arr
        for b, arr in enumerate(row_blocks(Ts)):
            m[f"tr{b}"] = arr
        for b, arr in enumerate(row_blocks(Xs)):
            m[f"xr{b}"] = arr
        # coef in column layout: [p, ti] = coef[128*ti + p]
        m["coefc"] = np.ascontiguousarray(cs.reshape(NTC, P).T).astype(f8)
        in_maps.append(m)
    return in_maps


def _get_callable():
    """Cached (fn, in_names, out_names, out_avals, zero_outs, mesh) for the
    sharded 8-core NEFF execution."""
    if "call" in _CACHE:
        return _CACHE["call"]

    import jax
    from jax.sharding import Mesh, PartitionSpec
    from jax.experimental.shard_map import shard_map

    import concourse.mybir as mybir
    from concourse import bass2jax
    from concourse.bass2jax import install_neuronx_cc_hook

    install_neuronx_cc_hook()
    nc = _get_program()

    partition_name = (
        nc.partition_id_tensor.name if nc.partition_id_tensor else None
    )
    in_names, out_names, out_avals, zero_outs = [], [], [], []
    for alloc in nc.m.functions[0].allocations:
        if not isinstance(alloc, mybir.MemoryLocationSet):
            continue
        if alloc.kind not in ("ExternalInput", "ExternalOutput"):
            continue
        name = alloc.memorylocations[0].name
        if alloc.kind == "ExternalInput":
            if name != partition_name:
                in_names.append(name)
        else:
            out_names.append(name)
            shape = tuple(alloc.tensor_shape)
            dtype = mybir.dt.np(alloc.dtype)
            out_avals.append(jax.core.ShapedArray(shape, dtype))
            zero_outs.append(np.zeros(shape, dtype))
    all_in_names = in_names + out_names
    if partition_name is not None:
        all_in_names = all_in_names + [partition_name]

    def _body(*args):
        operands = list(args)
        if partition_name is not None:
            operands.append(bass2jax.partition_id_tensor())
        outs = bass2jax._bass_exec_p.bind(
            *operands,
            out_avals=tuple(out_avals),
            in_names=tuple(all_in_names),
            out_names=tuple(out_names),
            lowering_input_output_aliases=(),
            sim_require_finite=True,
            sim_require_nnan=True,
            nc=nc,
        )
        return tuple(outs)

    devices = jax.devices()[:N_CORES]
    mesh = Mesh(np.asarray(devices), ("core",))
    n_all = len(in_names) + len(out_names)
    fn = jax.jit(
        shard_map(
            _body,
            mesh=mesh,
            in_specs=(PartitionSpec("core"),) * n_all,
            out_specs=(PartitionSpec("core"),) * len(out_names),
            check_rep=False,
        ),
        keep_unused=True,
    )
    _CACHE["call"] = (fn, in_names, out_names, out_avals, zero_outs, mesh)
    return _CACHE["call"]


def concat_inputs(in_maps):
    fn, in_names, out_names, out_avals, zero_outs, mesh = _get_callable()
    concat_in = [
        np.concatenate([np.asarray(m[name]) for m in in_maps], axis=0)
        for name in in_names
    ]
    concat_zeros = [
        np.zeros((N_CORES * z.shape[0], *z.shape[1:]), z.dtype) for z in zero_outs
    ]
    return concat_in + concat_zeros


def kernel(X, train_X, dual_coef):
    X = np.asarray(X, dtype=np.float32)
    train_X = np.asarray(train_X, dtype=np.float32)
    dual_coef = np.asarray(dual_coef, dtype=np.float32)

    fn, in_names, out_names, out_avals, zero_outs, mesh = _get_callable()
    in_maps = make_in_maps(X, train_X, dual_coef)
    args = concat_inputs(in_maps)
    outs = fn(*args)
    # per-core partials: core c = (qgroup i = c//TG, train half j = c%TG)
    parts = np.asarray(outs[0]).reshape(QG, TG, QPC)
    out = parts.sum(axis=1).reshape(-1)
    return out.astype(np.float32)


# revision 30
# speedup vs baseline: 1.0064x; 1.0064x over previous
"""RBF kernel ridge regression inference on 8 Trainium2 NeuronCores.

out[q] = sum_t exp(-gamma * ||X[q] - T[t]||^2) * coef[t]
       = exp(-g*x2[q]) * sum_t exp(2g*dot[t,q] - g*y2[t]) * coef[t]

Train index on PSUM partitions, query index on the free axis, all GEMM data
in fp8:

  TensorE: dot GEMM in fp8 DoubleRow mode (256-deep contraction per matmul,
           2x PE throughput), 8 matmuls per [128,2048] psum tile, preceded
           by HAM-warmup matmuls that run while the input DMAs land; then a
           coef-matvec sweep over the resident fp8 exp tiles, 4 col-tiled
           matmuls at a time running concurrently in separate 32-column
           groups of the PE array, accumulating into four [1,512] S rows
           packed at partitions 0/32/64/96 of one psum bank.
  ScalarE: one fused exp per [128,2048] psum tile (bias = -g*y2[t] per
           partition, scale = 2g), writing fp8 et tiles that stay resident
           in SBUF (64KB/partition) until the matvec sweep.
  DVE:     row norms only (square+accumulate at 1 elem/cycle/lane), plus
           the tiny row-space epilogue multiply by exp(-g*x2[q]).
  DMA:     x/tt on the SP hardware queue (first-needed slices first),
           norm inputs on the gpsimd software queue; x2 column->row
           transpose bounced through DRAM under phase 1.

Sharding: 4 query-groups x 2 train-groups over 8 cores; host sums the two
train partials per query group (32KB of adds).
"""

import numpy as np
import ml_dtypes

GAMMA = 1.0
N_QUERY, N_TRAIN, D = 8192, 8192, 512
N_CORES = 8
P = 128
QG, TG = 4, 2             # query groups x train groups = 8 cores
QPC = N_QUERY // QG       # 2048 queries per core
TPC = N_TRAIN // TG       # 4096 train points per core
NQC = QPC // P            # 16 query chunks of 128
NTC = TPC // P            # 32 train chunks of 128
NDR = 2                   # DoubleRow contraction groups (2 x 256 = 512)
NPAIR = NTC // 2          # 16 train-chunk pairs for the DR matvec
NJ = QPC // 512           # 4 matvec/qcol slices

_CACHE = {}


def _build_program():
    from contextlib import ExitStack

    import concourse.bass as bass
    import concourse.mybir as mybir
    import concourse.tile as tile
    from concourse import bacc

    f32 = mybir.dt.float32
    bf16 = mybir.dt.bfloat16
    f8 = mybir.dt.float8e4
    AF = mybir.ActivationFunctionType
    MUL = mybir.AluOpType.mult
    DR = mybir.MatmulPerfMode.DoubleRow

    nc = bacc.Bacc(
        "TRN2", target_bir_lowering=False, debug=False, num_devices=N_CORES
    )

    # DRAM inputs (host-pre-laid-out; see make_in_maps)
    tt_d = [
        nc.dram_tensor(f"tt{g}", [P, NDR, TPC], f8, kind="ExternalInput").ap()
        for g in range(NDR)
    ]
    x_d = [
        nc.dram_tensor(f"x{g}", [P, NDR, QPC], f8, kind="ExternalInput").ap()
        for g in range(NDR)
    ]
    tr_d = [
        nc.dram_tensor(f"tr{b}", [P, 8, D], f8, kind="ExternalInput").ap()
        for b in range(TPC // (8 * P))
    ]
    xr_d = [
        nc.dram_tensor(f"xr{b}", [P, 8, D], f8, kind="ExternalInput").ap()
        for b in range(QPC // (8 * P))
    ]
    coef_d = nc.dram_tensor("coefc", [P, NTC], f8, kind="ExternalInput").ap()
    out_d = nc.dram_tensor("out", [QPC], f32, kind="ExternalOutput").ap()
    x2_d = nc.dram_tensor("x2_bounce", [QPC], f32).ap()   # internal scratch

    with tile.TileContext(nc) as tc, ExitStack() as ctx:
        res = ctx.enter_context(tc.tile_pool(name="res", bufs=1))
        stream = ctx.enter_context(tc.tile_pool(name="stream", bufs=2))

        # ---- resident GEMM operands ----
        tt_sb, x_sb = [], []
        for g in range(NDR):
            t = res.tile([P, NDR, QPC], f8, tag=f"x{g}", name=f"x{g}s")
            x_sb.append(t)
        for g in range(NDR):
            t = res.tile([P, NDR, TPC], f8, tag=f"tt{g}", name=f"tt{g}s")
            tt_sb.append(t)
        # first-needed slices first: one x j-slice per group + the first
        # 512 train columns, then the bulk
        for g in range(NDR):
            nc.sync.dma_start(x_sb[g][:, :, :512], x_d[g][:, :, :512])
        for g in range(NDR):
            nc.sync.dma_start(tt_sb[g][:, :, :512], tt_d[g][:, :, :512])
        for joff in range(512, QPC, 512):
            for g in range(NDR):
                nc.sync.dma_start(
                    x_sb[g][:, :, joff : joff + 512],
                    x_d[g][:, :, joff : joff + 512],
                )
        QTR = TPC // 4
        for g in range(NDR):
            nc.sync.dma_start(tt_sb[g][:, :, 512:QTR], tt_d[g][:, :, 512:QTR])

        # ---- norms: ny2 = -g*||T[t]||^2 (col, bias), nx2 = -g*||X[q]||^2 ----
        ny2 = res.tile([P, NTC], f32, tag="ny2")
        trt0 = stream.tile([P, 8, D], f8, tag="tr", name="trt0")
        nc.gpsimd.dma_start(trt0[:], tr_d[0])
        coefc = res.tile([P, NTC], f8, tag="coefc")
        nc.gpsimd.dma_start(coefc[:], coef_d)
        for h in range(1, 4):
            for g in range(NDR):
                sl = slice(h * QTR, (h + 1) * QTR)
                nc.sync.dma_start(tt_sb[g][:, :, sl], tt_d[g][:, :, sl])
        for b in range(len(tr_d)):
            if b == 0:
                trt = trt0
            else:
                trt = stream.tile([P, 8, D], f8, tag="tr")
                nc.gpsimd.dma_start(trt[:], tr_d[b])
            tscr = stream.tile([P, 8, D], bf16, tag="scr", name="tscr")
            for j in range(8):
                nc.vector.scalar_tensor_tensor(
                    tscr[:, j : j + 1, :],
                    trt[:, j : j + 1, :],
                    -GAMMA,
                    trt[:, j : j + 1, :],
                    MUL,
                    MUL,
                    accum_out=ny2[:, 8 * b + j : 8 * b + j + 1],
                )
        nx2 = res.tile([P, NQC], f32, tag="nx2")
        for b in range(len(xr_d)):
            xrt = stream.tile([P, 8, D], f8, tag="xr")
            nc.gpsimd.dma_start(xrt[:], xr_d[b])
            xscr = stream.tile([P, 8, D], bf16, tag="scr", name="xscr")
            for j in range(8):
                nc.vector.scalar_tensor_tensor(
                    xscr[:, j : j + 1, :],
                    xrt[:, j : j + 1, :],
                    -GAMMA,
                    xrt[:, j : j + 1, :],
                    MUL,
                    MUL,
                    accum_out=nx2[:, 8 * b + j : 8 * b + j + 1],
                )
        # x2 -> row layout early (scattered bounce hides under phase 1)
        nc.sync.dma_start(x2_d.rearrange("(c p) -> p c", p=P), nx2[:])
        nx2_4 = res.tile([P, 512], f32, tag="nx2_4")
        for j in range(NJ):
            nc.sync.dma_start(
                nx2_4[32 * j : 32 * j + 1, :],
                x2_d[j * 512 : (j + 1) * 512].rearrange("(a q) -> a q", a=1),
            )

        # ---- phase 1: GEMM + exp, double-buffered [128,2048] psum tiles;
        # all 32 et tiles stay resident in SBUF for phase 2 ----
        ET = res.tile([P, NTC * QPC], f8, tag="ET")
        psq = ctx.enter_context(tc.tile_pool(name="psq", bufs=2, space="PSUM"))
        if True:
            # HAM warmup: dummy matmuls on the first-arrived slices run during
            # the remaining input DMAs so phase 1 starts at the full PE clock
            wu = psq.tile([P, QPC], f32, tag="ps", name="wu")
            for w in range(8):
                nc.tensor.matmul(
                    wu[:, (w % 4) * 512 : (w % 4) * 512 + 512],
                    tt_sb[0][:, :, :P],
                    x_sb[0][:, :, :512],
                    start=True,
                    stop=True,
                    perf_mode=DR,
                )
            for ti in range(NTC):
                ps = psq.tile([P, QPC], f32, tag="ps")
                for joff in range(0, QPC, 512):
                    for g in range(NDR):
                        nc.tensor.matmul(
                            ps[:, joff : joff + 512],
                            tt_sb[g][:, :, ti * P : (ti + 1) * P],
                            x_sb[g][:, :, joff : joff + 512],
                            start=(g == 0),
                            stop=(g == NDR - 1),
                            perf_mode=DR,
                        )
                nc.scalar.activation(
                    ET[:, ti * QPC : (ti + 1) * QPC],
                    ps[:],
                    AF.Exp,
                    bias=ny2[:, ti : ti + 1],
                    scale=2.0 * GAMMA,
                )

        ex2_4 = res.tile([P, 512], f32, tag="ex2_4")
        for j in range(NJ):
            nc.scalar.activation(
                ex2_4[32 * j : 32 * j + 1, :], nx2_4[32 * j : 32 * j + 1, :], AF.Exp
            )
        S4 = psq.tile([P, 512], f32, tag="ps", name="S4")

        # ---- phase 2: coef-matvec sweep; per ti the 4 col-tiled fp8
        # matmuls run concurrently in separate 32-column groups ----
        for ti in range(NTC):
            for j in range(NJ):
                nc.tensor.matmul(
                    S4[32 * j : 32 * j + 1, :],
                    coefc[:, ti : ti + 1],
                    ET[:, ti * QPC + j * 512 : ti * QPC + (j + 1) * 512],
                    start=(ti == 0),
                    stop=(ti == NTC - 1),
                    tile_position=(0, 32 * j),
                )

        # ---- epilogue: out rows = exp(-g*x2) * S, multiplied in place at
        # partitions 0/32/64/96 (DVE reads PSUM directly), then 4 contiguous
        # out DMAs ----
        out4 = res.tile([P, 512], f32, tag="out4")
        for j in range(NJ):
            nc.vector.tensor_mul(
                out4[32 * j : 32 * j + 1, :],
                S4[32 * j : 32 * j + 1, :],
                ex2_4[32 * j : 32 * j + 1, :],
            )
            nc.sync.dma_start(
                out_d[j * 512 : (j + 1) * 512].rearrange("(a q) -> a q", a=1),
                out4[32 * j : 32 * j + 1, :],
            )

    nc.compile()
    return nc


def _get_program():
    if "nc" not in _CACHE:
        _CACHE["nc"] = _build_program()
    return _CACHE["nc"]


def make_in_maps(X, train_X, dual_coef):
    bf = ml_dtypes.bfloat16
    f8 = ml_dtypes.float8_e4m3

    def dr_layout(A):
        # A: [n, D] rows -> [g][P, 2, n] with d = 256*g + 128*i + p
        At = np.ascontiguousarray(A.T).reshape(NDR, 2, P, A.shape[0])
        return [np.ascontiguousarray(At[g].transpose(1, 0, 2)).astype(f8)
                for g in range(NDR)]

    def row_blocks(A):
        # A: [n, D] -> [n//(8P)][P, 8, D] with row = b*8P + j*P + p
        n = A.shape[0]
        R = A.reshape(n // (8 * P), 8, P, D)
        return [np.ascontiguousarray(R[b].transpose(1, 0, 2)).astype(f8)
                for b in range(n // (8 * P))]

    in_maps = []
    for c in range(N_CORES):
        i, j = c // TG, c % TG
        Xs = X[i * QPC : (i + 1) * QPC]
        Ts = train_X[j * TPC : (j + 1) * TPC]
        cs = dual_coef[j * TPC : (j + 1) * TPC]
        m = {}
        for g, arr in enumerate(dr_layout(Xs)):
            m[f"x{g}"] = arr
        for g, arr in enumerate(dr_layout(Ts)):
            m[f"tt{g}"] = arr
        for b, arr in enumerate(row_blocks(Ts)):
            m[f"tr{b}"] = arr
        for b, arr in enumerate(row_blocks(Xs)):
            m[f"xr{b}"] = arr
        # coef in column layout: [p, ti] = coef[128*ti + p]
        m["coefc"] = np.ascontiguousarray(cs.reshape(NTC, P).T).astype(f8)
        in_maps.append(m)
    return in_maps


def _get_callable():
    """Cached (fn, in_names, out_names, out_avals, zero_outs, mesh) for the
    sharded 8-core NEFF execution."""
    if "call" in _CACHE:
        return _CACHE["call"]

    import jax
    from jax.sharding import Mesh, PartitionSpec
    from jax.experimental.shard_map import shard_map

    import concourse.mybir as mybir
    from concourse import bass2jax
    from concourse.bass2jax import install_neuronx_cc_hook

    install_neuronx_cc_hook()
    nc = _get_program()

    partition_name = (
        nc.partition_id_tensor.name if nc.partition_id_tensor else None
    )
    in_names, out_names, out_avals, zero_outs = [], [], [], []
    for alloc in nc.m.functions[0].allocations:
        if not isinstance(alloc, mybir.MemoryLocationSet):
            continue
        if alloc.kind not in ("ExternalInput", "ExternalOutput"):
            continue
        name = alloc.memorylocations[0].name
        if alloc.kind == "ExternalInput":
            if name != partition_name:
                in_names.append(name)
        else:
            out_names.append(name)
            shape = tuple(alloc.tensor_shape)
            dtype = mybir.dt.np(alloc.dtype)
            out_avals.append(jax.core.ShapedArray(shape, dtype))
            zero_outs.append(np.zeros(shape, dtype))
    all_in_names = in_names + out_names
    if partition_name is not None:
        all_in_names = all_in_names + [partition_name]

    def _body(*args):
        operands = list(args)
        if partition_name is not None:
            operands.append(bass2jax.partition_id_tensor())
        outs = bass2jax._bass_exec_p.bind(
            *operands,
            out_avals=tuple(out_avals),
            in_names=tuple(all_in_names),
            out_names=tuple(out_names),
            lowering_input_output_aliases=(),
            sim_require_finite=True,
            sim_require_nnan=True,
            nc=nc,
        )
        return tuple(outs)

    devices = jax.devices()[:N_CORES]
    mesh = Mesh(np.asarray(devices), ("core",))
    n_all = len(in_names) + len(out_names)
    fn = jax.jit(
        shard_map(
            _body,
            mesh=mesh,
            in_specs=(PartitionSpec("core"),) * n_all,
            out_specs=(PartitionSpec("core"),) * len(out_names),
            check_rep=False,
        ),
        keep_unused=True,
    )
    _CACHE["call"] = (fn, in_names, out_names, out_avals, zero_outs, mesh)
    return _CACHE["call"]


def concat_inputs(in_maps):
    fn, in_names, out_names, out_avals, zero_outs, mesh = _get_callable()
    concat_in = [
        np.concatenate([np.asarray(m[name]) for m in in_maps], axis=0)
        for name in in_names
    ]
    concat_zeros = [
        np.zeros((N_CORES * z.shape[0], *z.shape[1:]), z.dtype) for z in zero_outs
    ]
    return concat_in + concat_zeros


def kernel(X, train_X, dual_coef):
    X = np.asarray(X, dtype=np.float32)
    train_X = np.asarray(train_X, dtype=np.float32)
    dual_coef = np.asarray(dual_coef, dtype=np.float32)

    fn, in_names, out_names, out_avals, zero_outs, mesh = _get_callable()
    in_maps = make_in_maps(X, train_X, dual_coef)
    args = concat_inputs(in_maps)
    outs = fn(*args)
    # per-core partials: core c = (qgroup i = c//TG, train half j = c%TG)
    parts = np.asarray(outs[0]).reshape(QG, TG, QPC)
    out = parts.sum(axis=1).reshape(-1)
    return out.astype(np.float32)
